# revision 33
# baseline (speedup 1.0000x reference)
"""Trainium2 Bass kernel for nn_BoundaryPredictor1 (segment_reduce).

Per-core work (data-parallel over batch, 1 row per core):
  hidden [4096, 768] -> MLP logits -> +logistic noise -> hard boundary bits
  -> segment ids (cumsum) -> segment mean-pool -> pooled [4096, 768].

Implementation outline:
  A) bf16 MLP: convert+transpose hidden chunks, W1/W2 stationary, logits row.
  B) borderline-token fixup: tokens with |logits+noise| < delta recomputed
     with hi/lo bf16-pair matmuls (near-fp32 logits), scattered back.
  C) seg = exclusive cumsum of hard bits (two-level scan).
  D) pooling: per 128-token chunk, build one-hot indicator IndT[t, s] =
     (seg[t] == s + 128*r0) on the fly, matmul IndT.T @ hidden_bf to get
     per-chunk segment partial sums in PSUM, accumulate into a bf16
     accumulator at dynamic block offset r0; a ones column fused into the
     rhs yields counts. Finally scale by 1/max(cnt,1) and DMA out.
"""
import os
from contextlib import ExitStack

import numpy as np

import concourse.bass as bass
import concourse.bacc as bacc
import concourse.tile as tile
from concourse import mybir
from concourse._compat import with_exitstack
from concourse.bass import ds
from concourse.masks import make_identity

P = 128
L = 4096
D = 768
H = 512
B = 8
NCHUNK = 32          # 4096 / 128
NGROUP = 8           # 4096 / 512
DELTA2 = 0.02 * 0.02  # |x| < 0.02 -> recompute in high precision
BIGIDX = 1 << 20      # scatter index for dropped (padded) fixup slots

F32 = mybir.dt.float32
BF16 = mybir.dt.bfloat16
I32 = mybir.dt.int32
U32 = mybir.dt.uint32
Alu = mybir.AluOpType
Act = mybir.ActivationFunctionType


KSTAGE = int(os.environ.get("KSTAGE", "4"))
XSUB = int(os.environ.get("XSUB", "9"))


@with_exitstack
def boundary_kernel(ctx: ExitStack, tc: tile.TileContext, outs, ins):
    nc = tc.nc
    hidden = ins["hidden"]      # [4096, 768] f32 DRAM
    noise = ins["noise"]        # [4096, 1] f32 DRAM
    W1 = ins["W1"]              # [768, 512]
    b1 = ins["b1"]              # [512, 1]
    W2 = ins["W2"]              # [512, 1]
    b2 = ins["b2"]              # [1, 1]
    pooled = outs["pooled"]     # [4096, 768] f32
    nb_out = outs["nb"]         # [1, 1] f32

    # internal DRAM scratch
    hard_buf = nc.dram_tensor("hard_buf", [L, 1], F32, kind="Internal").ap()
    seg_buf = nc.dram_tensor("seg_buf", [L, 1], F32, kind="Internal").ap()
    logits_buf = nc.dram_tensor("logits_buf", [L, 1], F32, kind="Internal").ap()

    persist = ctx.enter_context(tc.tile_pool(name="persist", bufs=1))
    temps = ctx.enter_context(tc.tile_pool(name="temps", bufs=2))

    # ---------------- constants ----------------
    ident_bf = persist.tile([P, P], BF16)
    ident_f = persist.tile([P, P], F32)
    make_identity(nc, ident_f)
    nc.vector.tensor_copy(ident_bf, ident_f)

    iota_sf_i = persist.tile([P, P], I32)   # value = free index s (all partitions)
    nc.gpsimd.iota(iota_sf_i, pattern=[[1, P]], base=0, channel_multiplier=0)
    iota_sf = persist.tile([P, P], F32)
    nc.vector.tensor_copy(iota_sf, iota_sf_i)

    iota_col_i = persist.tile([P, 1], I32)  # value = partition index p
    nc.gpsimd.iota(iota_col_i, pattern=[[1, 1]], base=0, channel_multiplier=1)
    iota_col = persist.tile([P, 1], F32)
    nc.vector.tensor_copy(iota_col, iota_col_i)

    # strictly-upper triangular (k < m) for exclusive partition-prefix
    ut_strict = persist.tile([P, P], F32)
    nc.vector.tensor_scalar(out=ut_strict, in0=iota_sf, scalar1=iota_col,
                            scalar2=0.0, op0=Alu.subtract, op1=Alu.is_gt)

    iota_w1_i = persist.tile([16, 256], I32)  # wrapped token id + 1
    nc.gpsimd.iota(iota_w1_i, pattern=[[16, 256]], base=1, channel_multiplier=1)
    iota_w1 = persist.tile([16, 256], F32)
    nc.vector.tensor_copy(iota_w1, iota_w1_i)

    # ---------------- weights ----------------
    w1f_stage = temps.tile([P, 6, H], F32, tag="w1stage")
    nc.sync.dma_start(w1f_stage, W1.rearrange("(k p) h -> p k h", p=P))
    w1bf = persist.tile([P, 6, H], BF16)
    nc.scalar.copy(w1bf, w1f_stage)
    w1lo = persist.tile([P, 6, H], BF16)
    w1hi_f = temps.tile([P, 6, H], F32, tag="w1stage")
    nc.vector.tensor_copy(w1hi_f, w1bf)
    nc.vector.tensor_tensor(out=w1hi_f, in0=w1f_stage, in1=w1hi_f, op=Alu.subtract)
    nc.scalar.copy(w1lo, w1hi_f)

    w2f = persist.tile([P, 4], F32)
    nc.sync.dma_start(w2f, W2.rearrange("(m p) o -> p (m o)", p=P))
    w2bf = persist.tile([P, 4], BF16)
    nc.scalar.copy(w2bf, w2f)
    w2lo = persist.tile([P, 4], BF16)
    w2t = persist.tile([P, 4], F32)
    nc.vector.tensor_copy(w2t, w2bf)
    nc.vector.tensor_tensor(out=w2t, in0=w2f, in1=w2t, op=Alu.subtract)
    nc.scalar.copy(w2lo, w2t)

    b1col = persist.tile([P, 4], F32)
    nc.sync.dma_start(b1col, b1.rearrange("(m p) o -> p (m o)", p=P))
    b2t = persist.tile([1, 1], F32)
    nc.sync.dma_start(b2t, b2)

    # ---------------- persistent big buffers ----------------
    hidden_bf = persist.tile([P, NCHUNK, D + 1], BF16)   # col 768 = ones
    nc.gpsimd.memset(hidden_bf[:, :, D:D + 1], 1.0)
    logits_row = persist.tile([1, L], F32)
    zt = persist.tile([P, D + 1], F32)                    # zeros for G init
    nc.vector.memset(zt, 0.0)

    # noise transforms (independent of logits; scheduler overlaps with phase A)
    noise32 = persist.tile([P, NCHUNK], F32)
    nc.sync.dma_start(noise32, noise.rearrange("(p f) o -> p (f o)", p=P))
    noise_l = persist.tile([P, NCHUNK], F32)
    t_ln = temps.tile([P, NCHUNK], F32, tag="xtmp")
    nc.scalar.activation(noise_l, noise32, Act.Ln)
    nc.scalar.activation(t_ln, noise32, Act.Ln, bias=1.0, scale=-1.0)
    nc.vector.tensor_tensor(out=noise_l, in0=noise_l, in1=t_ln, op=Alu.subtract)

    noise_w = persist.tile([16, 256], F32)
    nc.sync.dma_start(noise_w, noise.rearrange("(g q) o -> q (g o)", q=16))
    noise_lw = persist.tile([16, 256], F32)
    t_lnw = temps.tile([16, 256], F32, tag="xwtmp")
    nc.scalar.activation(noise_lw, noise_w, Act.Ln)
    nc.scalar.activation(t_lnw, noise_w, Act.Ln, bias=1.0, scale=-1.0)
    nc.vector.tensor_tensor(out=noise_lw, in0=noise_lw, in1=t_lnw, op=Alu.subtract)

    # ================ phase A: bf16 MLP ================
    with tc.tile_pool(name="psA", bufs=2, space="PSUM") as psA, \
         tc.tile_pool(name="mlpA", bufs=2) as mlpA, \
         tc.tile_pool(name="mlpA1", bufs=3) as mlpA1:
        for g in range(NGROUP):
            htT = mlpA.tile([P, 6, 512], BF16, tag="htT")
            for c4 in range(4):
                chunk = 4 * g + c4
                hchunk = mlpA1.tile([P, D], F32, tag="hchunk")
                nc.sync.dma_start(
                    hchunk, hidden[128 * chunk:128 * chunk + 128, :])
                nc.scalar.copy(hidden_bf[:, chunk, 0:D], hchunk)
                for k in range(6):
                    pst = psA.tile([P, P], BF16, tag="pst")
                    nc.tensor.transpose(
                        pst, hidden_bf[:, chunk, 128 * k:128 * k + 128], ident_bf)
                    nc.vector.tensor_copy(
                        htT[:, k, 128 * c4:128 * c4 + 128], pst)
            hrelu = mlpA.tile([P, 4, 512], BF16, tag="hrelu")
            for m in range(4):
                psz = psA.tile([P, 512], F32, tag="psz")
                for k in range(6):
                    nc.tensor.matmul(
                        psz, lhsT=w1bf[:, k, 128 * m:128 * m + 128],
                        rhs=htT[:, k, :], start=(k == 0), stop=(k == 5))
                nc.scalar.activation(hrelu[:, m, :], psz, Act.Relu,
                                     bias=b1col[:, m:m + 1], scale=1.0)
            psl = psA.tile([1, 512], F32, tag="psl")
            for m in range(4):
                nc.tensor.matmul(psl, lhsT=w2bf[:, m:m + 1], rhs=hrelu[:, m, :],
                                 start=(m == 0), stop=(m == 3))
            nc.scalar.activation(logits_row[0:1, 512 * g:512 * g + 512], psl,
                                 Act.Identity, bias=b2t[0:1, 0:1], scale=1.0)

    if KSTAGE < 2:
        return
    # ================ phase X: hard bits, fixup, seg ================
    with tc.tile_pool(name="psX", bufs=1, space="PSUM") as psX, \
         tc.tile_pool(name="xpool", bufs=1) as xp:
        # bounce logits through DRAM (single-partition SBUF->SBUF reshape DMAs
        # proved unreliable on HW)
        nc.sync.dma_start(logits_buf.rearrange("(a b) o -> a (b o)", a=1),
                          logits_row)
        # hard bits in [128, 32] layout (t = 32p + f)
        logits32 = xp.tile([P, NCHUNK], F32)
        nc.sync.dma_start(
            logits32, logits_buf.rearrange("(p f) o -> p (f o)", p=P))
        x32 = xp.tile([P, NCHUNK], F32)
        nc.vector.tensor_tensor(out=x32, in0=logits32, in1=noise_l, op=Alu.add)
        hard32 = xp.tile([P, NCHUNK], F32)
        nc.vector.tensor_scalar(out=hard32, in0=x32, scalar1=0.0, scalar2=None,
                                op0=Alu.is_gt)
        # full hard bits to DRAM (pre-fixup)
        nc.sync.dma_start(hard_buf.rearrange("(p f) o -> p (f o)", p=P), hard32)

        if XSUB < 2:
            return
        # --- flagged tokens in wrapped [16, 256] layout (t = q + 16g) ---
        logits_w = xp.tile([16, 256], F32)
        nc.sync.dma_start(logits_w,
                          logits_buf.rearrange("(g q) o -> q (g o)", q=16))
        x_w = xp.tile([16, 256], F32)
        nc.vector.tensor_tensor(out=x_w, in0=logits_w, in1=noise_lw, op=Alu.add)
        x2w = xp.tile([16, 256], F32)
        nc.vector.tensor_tensor(out=x2w, in0=x_w, in1=x_w, op=Alu.mult)
        # tok_or_neg = (x^2 < delta^2) * (t+1) - 1
        tok_or_neg = xp.tile([16, 256], F32)
        nc.vector.tensor_scalar(out=tok_or_neg, in0=x2w, scalar1=DELTA2,
                                scalar2=None, op0=Alu.is_lt)
        nc.vector.tensor_tensor(out=tok_or_neg, in0=tok_or_neg, in1=iota_w1,
                                op=Alu.mult)
        nc.vector.tensor_scalar(out=tok_or_neg, in0=tok_or_neg, scalar1=1.0,
                                scalar2=None, op0=Alu.subtract)
        fpos = xp.tile([16, 8], F32)          # up to 128 flagged tokens
        nfound = xp.tile([1, 1], U32)
        nc.gpsimd.sparse_gather(fpos, tok_or_neg, num_found=nfound)
        # flagged tokens' logistic noise, compacted in the same order:
        # enc = (x^2 < delta^2) * (noise_l + 1000) - 1
        nlw1k = xp.tile([16, 256], F32)
        nc.vector.tensor_scalar(out=nlw1k, in0=noise_lw, scalar1=1000.0,
                                scalar2=None, op0=Alu.add)
        nl_enc = xp.tile([16, 256], F32)
        nc.vector.scalar_tensor_tensor(out=nl_enc, in0=x2w, scalar=DELTA2,
                                       in1=nlw1k, op0=Alu.is_lt, op1=Alu.mult)
        nc.vector.tensor_scalar(out=nl_enc, in0=nl_enc, scalar1=1.0,
                                scalar2=None, op0=Alu.subtract)
        fnl_w = xp.tile([16, 8], F32)
        nf2 = xp.tile([1, 1], U32)
        nc.gpsimd.sparse_gather(fnl_w, nl_enc, num_found=nf2)

        # On HW, sparse_gather pads with ARBITRARY values (possibly NaN) past
        # num_found; sanitize with a count-based select (NaN-safe).
        iota88_i = xp.tile([16, 8], I32)
        nc.gpsimd.iota(iota88_i, pattern=[[16, 8]], base=0, channel_multiplier=1)
        iota88 = xp.tile([16, 8], F32)
        nc.vector.tensor_copy(iota88, iota88_i)
        nf_f = xp.tile([1, 1], F32)
        nc.vector.tensor_copy(nf_f, nfound)
        nf_bc = xp.tile([16, 1], F32)
        nc.gpsimd.partition_broadcast(nf_bc, nf_f, channels=16)
        vmask = xp.tile([16, 8], I32)
        nc.vector.tensor_scalar(out=vmask, in0=iota88, scalar1=nf_bc,
                                scalar2=None, op0=Alu.is_lt)
        negones = xp.tile([16, 8], F32)
        nc.vector.memset(negones, -1.0)
        fposc = xp.tile([16, 8], F32)
        nc.vector.select(fposc, vmask, fpos, negones)
        fnlc = xp.tile([16, 8], F32)
        nc.vector.select(fnlc, vmask, fnl_w, negones)

        # gather idx (clamped to [0, L-1]) and scatter idx (OOB for pads)
        gidx_w = xp.tile([16, 8], I32)
        tmpw = xp.tile([16, 8], F32)
        nc.vector.tensor_scalar(out=tmpw, in0=fposc, scalar1=0.0,
                                scalar2=float(L - 1), op0=Alu.max, op1=Alu.min)
        nc.vector.tensor_copy(gidx_w, tmpw)
        sidx_w = xp.tile([16, 8], I32)
        # fposc + (fposc < 0) * BIGIDX
        tmps = xp.tile([16, 8], F32)
        nc.vector.tensor_scalar(out=tmps, in0=fposc, scalar1=0.0,
                                scalar2=float(BIGIDX), op0=Alu.is_lt,
                                op1=Alu.mult)
        nc.vector.tensor_tensor(out=tmps, in0=tmps, in1=fposc, op=Alu.add)
        nc.vector.tensor_copy(sidx_w, tmps)

        # bounce wrapped [16, 8] -> linear [128, 1] via DRAM scratch
        fidx_buf = nc.dram_tensor("fidx_buf", [P, 1], I32, kind="Internal").ap()
        sidx_buf = nc.dram_tensor("sidx_buf", [P, 1], I32, kind="Internal").ap()
        fnl_buf = nc.dram_tensor("fnl_buf", [P, 1], F32, kind="Internal").ap()
        nc.sync.dma_start(fidx_buf.rearrange("(g q) o -> q (g o)", q=16), gidx_w)
        nc.sync.dma_start(sidx_buf.rearrange("(g q) o -> q (g o)", q=16), sidx_w)
        nc.sync.dma_start(fnl_buf.rearrange("(g q) o -> q (g o)", q=16), fnlc)
        gidx = xp.tile([P, 1], I32)
        sidx = xp.tile([P, 1], I32)
        fnl = xp.tile([P, 1], F32)
        nc.sync.dma_start(gidx, fidx_buf)
        nc.sync.dma_start(sidx, sidx_buf)
        nc.sync.dma_start(fnl, fnl_buf)
        nc.vector.tensor_scalar(out=fnl, in0=fnl, scalar1=999.0, scalar2=None,
                                op0=Alu.subtract)

        if XSUB < 3:
            return
        # gather hidden rows of flagged tokens (one row per partition)
        fhid = xp.tile([P, D], F32)
        nc.gpsimd.indirect_dma_start(
            out=fhid, out_offset=None, in_=hidden,
            in_offset=bass.IndirectOffsetOnAxis(ap=gidx[:, :1], axis=0))

        if XSUB < 4:
            return
        # hi/lo split of gathered rows
        fhi = xp.tile([P, D], BF16)
        nc.scalar.copy(fhi, fhid)
        flo = xp.tile([P, D], BF16)
        fres = xp.tile([P, D], F32)
        nc.vector.tensor_copy(fres, fhi)
        nc.vector.tensor_tensor(out=fres, in0=fhid, in1=fres, op=Alu.subtract)
        nc.scalar.copy(flo, fres)
        # transpose both
        fT = xp.tile([P, 6, 2, P], BF16)
        for k in range(6):
            for j, src in enumerate((fhi, flo)):
                pst = psX.tile([P, P], BF16, tag="pstX")
                nc.tensor.transpose(pst, src[:, 128 * k:128 * k + 128], ident_bf)
                nc.vector.tensor_copy(fT[:, k, j, :], pst)
        # L1: z = hi@Whi + lo@Whi + hi@Wlo
        frelu_f = xp.tile([P, 4, P], F32)
        frelu_hi = xp.tile([P, 4, P], BF16)
        frelu_lo = xp.tile([P, 4, P], BF16)
        for m in range(4):
            psz = psX.tile([P, P], F32, tag="pszX")
            for k in range(6):
                nc.tensor.matmul(psz, lhsT=w1bf[:, k, 128 * m:128 * m + 128],
                                 rhs=fT[:, k, 0, :], start=(k == 0), stop=False)
                nc.tensor.matmul(psz, lhsT=w1bf[:, k, 128 * m:128 * m + 128],
                                 rhs=fT[:, k, 1, :], start=False, stop=False)
                nc.tensor.matmul(psz, lhsT=w1lo[:, k, 128 * m:128 * m + 128],
                                 rhs=fT[:, k, 0, :], start=False,
                                 stop=(k == 5))
            nc.scalar.activation(frelu_f[:, m, :], psz, Act.Relu,
                                 bias=b1col[:, m:m + 1], scale=1.0)
            nc.scalar.copy(frelu_hi[:, m, :], frelu_f[:, m, :])
        # lo = bf16(relu - f32(hi))
        hi_f = xp.tile([P, 4, P], F32)
        nc.vector.tensor_copy(hi_f, frelu_hi)
        nc.vector.tensor_tensor(out=hi_f, in0=frelu_f, in1=hi_f,
                                op=Alu.subtract)
        nc.scalar.copy(frelu_lo, hi_f)

        psl = psX.tile([1, P], F32, tag="pslX")
        for m in range(4):
            nc.tensor.matmul(psl, lhsT=w2bf[:, m:m + 1], rhs=frelu_hi[:, m, :],
                             start=(m == 0), stop=False)
            nc.tensor.matmul(psl, lhsT=w2lo[:, m:m + 1], rhs=frelu_hi[:, m, :],
                             start=False, stop=False)
            nc.tensor.matmul(psl, lhsT=w2bf[:, m:m + 1], rhs=frelu_lo[:, m, :],
                             start=False, stop=(m == 3))
        flog_row = xp.tile([1, P], F32)
        nc.scalar.activation(flog_row, psl, Act.Identity, bias=b2t[0:1, 0:1],
                             scale=1.0)
        # transpose -> [128, 1]
        psf = psX.tile([P, 1], F32, tag="psfX")
        nc.tensor.transpose(psf, flog_row, ident_f[0:1, 0:1])
        fcol = xp.tile([P, 1], F32)
        nc.vector.tensor_copy(fcol, psf)
        # hard bit for flagged tokens
        nc.vector.tensor_tensor(out=fcol, in0=fcol, in1=fnl, op=Alu.add)
        fhard = xp.tile([P, 1], F32)
        nc.vector.tensor_scalar(out=fhard, in0=fcol, scalar1=0.0, scalar2=None,
                                op0=Alu.is_gt)
        # scatter corrected bits into hard_buf (padded slots dropped as OOB)
        nc.gpsimd.indirect_dma_start(
            out=hard_buf,
            out_offset=bass.IndirectOffsetOnAxis(ap=sidx[:, :1], axis=0),
            in_=fhard, in_offset=None,
            bounds_check=L - 1, oob_is_err=False)

        if XSUB < 5:
            return
        # --- seg ids ---
        hard32b = xp.tile([P, NCHUNK], F32)
        nc.sync.dma_start(hard32b,
                          hard_buf.rearrange("(p f) o -> p (f o)", p=P))
        zeros32 = xp.tile([P, NCHUNK], F32)
        nc.vector.memset(zeros32, 0.0)
        incl = xp.tile([P, NCHUNK], F32)
        nc.vector.tensor_tensor_scan(incl, hard32b, zeros32, 0.0,
                                     Alu.add, Alu.add)
        pscarry = psX.tile([P, 1], F32, tag="carryX")
        nc.tensor.matmul(pscarry, lhsT=ut_strict, rhs=incl[:, 31:32],
                         start=True, stop=True)
        carry = xp.tile([P, 1], F32)
        nc.vector.tensor_copy(carry, pscarry)
        inclc = xp.tile([P, NCHUNK], F32)
        nc.vector.tensor_scalar(out=inclc, in0=incl, scalar1=carry,
                                scalar2=None, op0=Alu.add)
        nc.sync.dma_start(nb_out, inclc[127:128, 31:32])
        seg32 = xp.tile([P, NCHUNK], F32)
        nc.vector.tensor_tensor(out=seg32, in0=inclc, in1=hard32b,
                                op=Alu.subtract)
        nc.sync.dma_start(seg_buf.rearrange("(p f) o -> p (f o)", p=P), seg32)

        if os.environ.get("KDEBUG"):
            with nc.allow_non_contiguous_dma(reason="debug"):
                nc.sync.dma_start(pooled[:, 0:1], hard_buf)
                nc.sync.dma_start(
                    pooled.rearrange("(a b) d -> a b d", a=P)[:, :, 1:2],
                    logits_row.rearrange("p (a b) -> p a b", a=P)[0:1]
                    .rearrange("p a b -> p a b ()"))
                nc.sync.dma_start(
                    pooled.rearrange("(a b) d -> a b d", a=P)[:, :, 2:3],
                    x32.rearrange("p f -> p f ()"))

        # --- per-chunk block bases ---
        seg_ct = xp.tile([32, P], F32)
        nc.sync.dma_start(seg_ct, seg_buf.rearrange("(c j) o -> c (j o)", c=32))
        # b0 = floor(base/128), robust to either convert rounding mode:
        # i1 = int(y); f1 = float(i1); b0 = f1 - (f1 > y)
        ycol = xp.tile([32, 1], F32)
        nc.vector.tensor_scalar(out=ycol, in0=seg_ct[:, 0:1],
                                scalar1=1.0 / 128.0, scalar2=None,
                                op0=Alu.mult)
        i1 = xp.tile([32, 1], I32)
        nc.vector.tensor_copy(i1, ycol)
        f1 = xp.tile([32, 1], F32)
        nc.vector.tensor_copy(f1, i1)
        corr = xp.tile([32, 1], F32)
        nc.vector.tensor_tensor(out=corr, in0=f1, in1=ycol, op=Alu.is_gt)
        b0f = xp.tile([32, 1], F32)
        nc.vector.tensor_tensor(out=b0f, in0=f1, in1=corr, op=Alu.subtract)
        r0128 = xp.tile([32, 1], F32)
        nc.vector.tensor_scalar(out=r0128, in0=b0f, scalar1=128.0,
                                scalar2=None, op0=Alu.mult)
        # 128*r0 as a [1, 32] row on partition 0, via DRAM bounce
        r0f_buf = nc.dram_tensor("r0f_buf", [32, 1], F32, kind="Internal").ap()
        nc.sync.dma_start(r0f_buf, r0128)
        r0row = persist.tile([1, 32], F32)
        nc.sync.dma_start(r0row, r0f_buf.rearrange("(a c) o -> a (c o)", a=1))
        segr_ct = xp.tile([32, P], F32)
        nc.vector.tensor_scalar(out=segr_ct, in0=seg_ct, scalar1=r0128,
                                scalar2=None, op0=Alu.subtract)
        pssg = psX.tile([P, 32], F32, tag="sgX")
        nc.tensor.transpose(pssg, segr_ct, ident_f[0:32, 0:32])
        segr_tp = persist.tile([P, 32], F32)
        nc.vector.tensor_copy(segr_tp, pssg)

    if KSTAGE < 3:
        return
    # ================ phase B: indicator matmuls + scatter-add to DRAM ======
    # Registers / dynamic-offset compute APs crash under Tile on this HW
    # path, so the accumulation uses only static APs plus indirect
    # scatter-add with per-partition row offsets (proven primitives).
    # G row s = raw segment sum (cols 0:768) and count (col 768).
    G = nc.dram_tensor("G", [33 * P, D + 1], F32, kind="Internal").ap()
    for w in range(33):
        nc.sync.dma_start(G[P * w:P * w + P, :], zt)

    with tc.tile_pool(name="psB", bufs=2, space="PSUM") as psB, \
         tc.tile_pool(name="bpool", bufs=3) as bp:
        for c in range(NCHUNK):
            pb0 = bp.tile([P, 1], F32, tag="pb0")
            nc.gpsimd.partition_broadcast(pb0, r0row[0:1, c:c + 1], channels=P)
            offsf = bp.tile([P, 2], F32, tag="offsf")
            nc.vector.tensor_scalar(out=offsf[:, 0:1], in0=pb0,
                                    scalar1=iota_col, scalar2=None,
                                    op0=Alu.add)
            nc.vector.tensor_scalar(out=offsf[:, 1:2], in0=offsf[:, 0:1],
                                    scalar1=128.0, scalar2=None, op0=Alu.add)
            offs = bp.tile([P, 2], I32, tag="offs")
            nc.vector.tensor_copy(offs, offsf)
            ind0 = bp.tile([P, P], BF16, tag="ind0")
            nc.vector.tensor_scalar(out=ind0, in0=iota_sf,
                                    scalar1=segr_tp[:, c:c + 1], scalar2=0.0,
                                    op0=Alu.subtract, op1=Alu.is_equal)
            ind1 = bp.tile([P, P], BF16, tag="ind1")
            nc.vector.tensor_scalar(out=ind1, in0=iota_sf,
                                    scalar1=segr_tp[:, c:c + 1], scalar2=-128.0,
                                    op0=Alu.subtract, op1=Alu.is_equal)
            for j, ind in enumerate((ind0, ind1)):
                pa = psB.tile([P, 384], F32, tag=f"pa{j}")
                nc.tensor.matmul(pa, lhsT=ind, rhs=hidden_bf[:, c, 0:384],
                                 start=True, stop=True)
                pb = psB.tile([P, 385], F32, tag=f"pb{j}")
                nc.tensor.matmul(pb, lhsT=ind, rhs=hidden_bf[:, c, 384:769],
                                 start=True, stop=True)
                stage = bp.tile([P, D + 1], F32, tag=f"stage{j}")
                nc.scalar.copy(stage[:, 0:384], pa)
                nc.scalar.copy(stage[:, 384:769], pb)
                nc.gpsimd.indirect_dma_start(
                    out=G,
                    out_offset=bass.IndirectOffsetOnAxis(
                        ap=offs[:, j:j + 1], axis=0),
                    in_=stage, in_offset=None,
                    compute_op=Alu.add)

    if KSTAGE < 4:
        return
    # ================ finalize: scale by 1/max(cnt,1), write out ================
    with tc.tile_pool(name="fpool", bufs=3) as fp:
        for w in range(NCHUNK):
            gt = fp.tile([P, D + 1], F32, tag="gt")
            nc.sync.dma_start(gt, G[P * w:P * w + P, :])
            cmax = fp.tile([P, 1], F32, tag="cmax")
            nc.vector.tensor_scalar(out=cmax, in0=gt[:, D:D + 1], scalar1=1.0,
                                    scalar2=None, op0=Alu.max)
            recip = fp.tile([P, 1], F32, tag="recip")
            nc.vector.reciprocal(recip, cmax)
            mask = fp.tile([P, 1], F32, tag="mask")
            nc.vector.tensor_scalar(out=mask, in0=gt[:, D:D + 1], scalar1=0.5,
                                    scalar2=None, op0=Alu.is_ge)
            nc.vector.tensor_tensor(out=recip, in0=recip, in1=mask,
                                    op=Alu.mult)
            stage = fp.tile([P, D], F32, tag="stage")
            nc.scalar.activation(stage, gt[:, 0:D], Act.Copy,
                                 scale=recip[:, 0:1])
            nc.sync.dma_start(pooled[128 * w:128 * w + 128, :], stage)


def build_nc():
    nc = bacc.Bacc("TRN2", target_bir_lowering=False, debug=False,
                   enable_asserts=False)
    ins = dict(
        hidden=nc.dram_tensor("hidden", [L, D], F32, kind="ExternalInput").ap(),
        noise=nc.dram_tensor("noise", [L, 1], F32, kind="ExternalInput").ap(),
        W1=nc.dram_tensor("W1", [D, H], F32, kind="ExternalInput").ap(),
        b1=nc.dram_tensor("b1", [H, 1], F32, kind="ExternalInput").ap(),
        W2=nc.dram_tensor("W2", [H, 1], F32, kind="ExternalInput").ap(),
        b2=nc.dram_tensor("b2", [1, 1], F32, kind="ExternalInput").ap(),
    )
    outs = dict(
        pooled=nc.dram_tensor("pooled", [L, D], F32, kind="ExternalOutput").ap(),
        nb=nc.dram_tensor("nb", [1, 1], F32, kind="ExternalOutput").ap(),
    )
    with tile.TileContext(nc) as tc:
        boundary_kernel(tc, outs, ins)
    nc.compile()
    return nc


_NC = None


def _get_nc():
    global _NC
    if _NC is None:
        _NC = build_nc()
    return _NC


def make_in_maps(hidden, W1, b1, W2, b2, noise_u):
    return [
        dict(
            hidden=np.ascontiguousarray(hidden[b], np.float32),
            noise=np.ascontiguousarray(noise_u[b].reshape(L, 1), np.float32),
            W1=np.ascontiguousarray(W1, np.float32),
            b1=np.ascontiguousarray(b1.reshape(H, 1), np.float32),
            W2=np.ascontiguousarray(W2, np.float32),
            b2=np.ascontiguousarray(b2.reshape(1, 1), np.float32),
        )
        for b in range(B)
    ]


def assemble(results):
    pooled = np.stack([r["pooled"] for r in results]).astype(np.float32)
    nb_total = float(sum(float(r["nb"][0, 0]) for r in results))
    ratio = nb_total / float(B * L)
    loss = max(abs(ratio - 0.25) - 0.05, 0.0)
    return (pooled, np.float32(loss), np.float32(nb_total),
            np.float32(float(B * L)))


def kernel(hidden, W1, b1, W2, b2, noise_u):
    from concourse import bass_utils
    nc = _get_nc()
    in_maps = make_in_maps(hidden, W1, b1, W2, b2, noise_u)
    res = bass_utils.run_bass_kernel_spmd(nc, in_maps, core_ids=list(range(B)))
    return assemble(res.results)
